# revision 11
# baseline (speedup 1.0000x reference)
"""DeepConvNet Trainium2 kernel.

3x [Conv3x3(pad=1) -> ReLU -> MaxPool2x2] -> Linear, N=64, input 3x128x128.

Sharding: pure data parallel, 8 images per NeuronCore across 8 cores.

Per-core dataflow (activations bf16 in SBUF, fp32 PSUM accumulation):
  conv1: im2col in partitions. 4-image groups, block-diagonal weights:
         K = 9 taps x 3 ch x 4 imgs = 108 partitions, M = 4 imgs x 32 ch.
         rhs built by ONE 4-D-AP DMA per (group, row-chunk) so conv1
         matmuls start as soon as the first chunk lands (just-in-time).
  pool:  PSUM laid out so each 2x2 quad is a [2,2] innermost block of a
         5-D view; a single DVE tensor_reduce(max, axis=XY) pools a whole
         [128,1024] PSUM tile, then one ScalarE activation applies
         relu(x+bias) while writing the zero-bordered padded tile.
         (bias is NOT folded into the matmul -- no ones rows.)
  conv2: kx-replicated rhs (K = 32 ch x 3 kx = 96) built by one 3-D-AP
         SBUF->SBUF DMA per image; 3 accumulated matmuls over ky; two
         images run concurrently via column tiling (img A -> array cols
         0-63, img B -> cols 64-127).
  conv3: no replication: 9 accumulated matmuls (K=64) per image; two
         images run concurrently via row tiling (A rows 0-63, B 64-127).
  fc:    256 accumulated matmuls (K=128 channels, one per spatial p),
         N = 8 images, M = 10 classes, 4-way column tiling.
"""

import os
import sys

import numpy as np

for _p in ("/opt/trn_rl_repo", "/root/.axon_site/_ro/trn_rl_repo"):
    if os.path.isdir(_p) and _p not in sys.path:
        sys.path.insert(0, _p)

import ml_dtypes

import concourse.bass as bass
import concourse.mybir as mybir
import concourse.tile as tile
from concourse import bacc
from concourse.bass_utils import run_bass_kernel_spmd

BF16 = mybir.dt.bfloat16
F32 = mybir.dt.float32
NPBF16 = ml_dtypes.bfloat16

N_CORES = 8
IMGS = 8          # images per core
GROUPS = 2        # conv1 image groups per core (4 imgs each)
G1 = 130          # conv1 padded width/height
W1WIN = 127 * G1 + 128  # flat window length per conv1 im2col row
W1ALLOC = 128 * G1
CHUNKS = 4        # im2col row-chunks per group (4 k-iters = 32 rows each)
P1 = 66           # conv1 pooled padded grid (64 + 2)
P1F = 67 * 66     # pp1 alloc free size (one guard row for the kx shifts)
P2 = 34           # conv2 pooled padded grid (32 + 2)
P2F = 34 * 34
MAX = mybir.AluOpType.max
ADD = mybir.AluOpType.add


def _build_nc(dbg=False):
    nc = bacc.Bacc("TRN2", target_bir_lowering=False, debug=False)

    xp = nc.dram_tensor("xp", [IMGS * 3 * G1 * G1], BF16, kind="ExternalInput")
    lhsT1 = nc.dram_tensor("lhsT1", [108, 128], BF16, kind="ExternalInput")
    lhsT2 = nc.dram_tensor("lhsT2", [96, 3 * 128], BF16, kind="ExternalInput")
    lhsT3 = nc.dram_tensor("lhsT3", [128, 9 * 128], BF16, kind="ExternalInput")
    wfc = nc.dram_tensor("wfc", [128, 2560], BF16, kind="ExternalInput")
    biases = nc.dram_tensor("biases", [128, 4], F32, kind="ExternalInput")
    scores = nc.dram_tensor("scores", [10, 8], F32, kind="ExternalOutput")

    Relu = mybir.ActivationFunctionType.Relu
    Ident = mybir.ActivationFunctionType.Identity
    Copy = mybir.ActivationFunctionType.Copy

    with tile.TileContext(nc) as tc:
        with (
            tc.tile_pool(name="wts", bufs=1) as wp,
            tc.tile_pool(name="rhs1", bufs=1) as rhs1p,
            tc.tile_pool(name="pp1", bufs=2) as pp1p,
            tc.tile_pool(name="rhs2", bufs=4) as rhs2p,
            tc.tile_pool(name="pp2", bufs=4) as pp2p,
            tc.tile_pool(name="xall", bufs=1) as xallp,
            tc.tile_pool(name="tmp", bufs=6) as tmpp,
            tc.tile_pool(name="ps", bufs=4, space="PSUM") as psp,
        ):
            # conv1 weights + biases first: conv1 start depends on them
            t_l1 = wp.tile([108, 128], BF16)
            nc.sync.dma_start(out=t_l1[:], in_=lhsT1.ap())
            t_b = wp.tile([128, 4], F32)
            nc.sync.dma_start(out=t_b[:], in_=biases.ap())

            # ---- im2col: one DMA per (row-chunk, tap), covering BOTH
            # groups (DMA APs may only step partitions in dim 0, so each
            # tap t needs its own trigger: dst partitions p = 9u + t).
            # Chunking by output rows lets conv1 start on chunk 0 while
            # later chunks are still landing.
            rhs1 = rhs1p.tile([108, 2 * W1ALLOC], BF16, name="rhs1")
            r1pitch = rhs1.ap[0][0]
            CW = (16 // CHUNKS) * 8 * G1  # chunk column width

            def im2col_chunk(c):
                col0 = c * CW
                wlen = CW if c < CHUNKS - 1 else W1WIN - col0
                for t in range(9):
                    a, b = divmod(t, 3)
                    src = bass.AP(
                        xp,
                        a * G1 + b + col0,
                        [[G1 * G1, 12], [12 * G1 * G1, 2], [1, wlen]],
                    )
                    dst = bass.AP(
                        rhs1.tensor,
                        rhs1.offset + t * r1pitch + col0,
                        [[9 * r1pitch, 12], [W1ALLOC, 2], [1, wlen]],
                    )
                    (nc.sync if (c * 9 + t) % 2 == 0 else nc.gpsimd).dma_start(
                        out=dst, in_=src
                    )

            # Only chunk 0 is triggered up front; chunks c>0 and the
            # late-stage weights are emitted inside the conv1 loop so the
            # tile scheduler's dependency batch for the first matmuls
            # covers chunk 0 alone. (The trigger queues run ahead of the
            # PE, so the DMAs still start early on the hardware.)
            im2col_chunk(0)
            t_l2 = wp.tile([96, 3 * 128], BF16)
            nc.scalar.dma_start(out=t_l2[:], in_=lhsT2.ap())

            if dbg:
                d_rhs1 = nc.dram_tensor(
                    "d_rhs1", [108, 2 * W1ALLOC], BF16, kind="ExternalOutput"
                )
                nc.sync.dma_start(out=d_rhs1.ap(), in_=rhs1[:])

            x_all = xallp.tile([128, 2048], BF16)

            def pool_psum(ps, w, out_ap, bias_col, name):
                """2x2 maxpool + bias + relu of a [128, 1024] PSUM tile
                holding rows of width w. ScalarE adds bias to even
                columns (PSUM->SBUF), DVE maxes odd+bias against it, DVE
                maxes row pairs (folding relu) directly into out_ap."""
                bias = t_b[:, bias_col : bias_col + 1]
                pv = ps.rearrange("p (rx dx) -> p rx dx", dx=2)
                e = tmpp.tile([128, 512], BF16, tag="tmp1", name=f"e_{name}")
                nc.scalar.activation(e[:], pv[:, :, 0], Ident, bias=bias)
                m = tmpp.tile([128, 512], BF16, tag="tmp2", name=f"m_{name}")
                nc.vector.scalar_tensor_tensor(
                    m[:], pv[:, :, 1], bias, e[:], ADD, MAX
                )
                a = m.rearrange("p (yo dy x) -> p yo dy x", dy=2, x=w // 2)
                nc.vector.scalar_tensor_tensor(
                    out_ap, a[:, :, 0, :], 0.0, a[:, :, 1, :], MAX, MAX
                )

            # =======================  conv1  =======================
            pp1_tiles = []
            rhs1g = rhs1.rearrange("p (g y x) -> p g y x", g=2, x=G1)
            for g in range(GROUPS):
                rhs1v = rhs1g[:, g]
                pp1 = pp1p.tile([128, P1F], BF16, tag="pp1", name=f"pp1_{g}")
                pv = pp1.rearrange("p (r q) -> p r q", q=P1)
                # zero borders + guard row only; interior is overwritten
                nc.gpsimd.memset(pp1[:, 0:P1], 0)
                nc.gpsimd.memset(pp1[:, 65 * P1 : P1F], 0)  # bottom + guard
                nc.gpsimd.memset(pv[:, 1:65, 0:1], 0)
                nc.gpsimd.memset(pv[:, 1:65, 65:66], 0)

                for k in range(16):
                    ps = psp.tile([128, 1024], F32, tag="ps", name=f"ps1_{g}_{k}")
                    for h in range(2):
                        y0 = k * 8 + h * 4
                        nc.tensor.matmul(
                            ps[:, h * 512 : (h + 1) * 512],
                            t_l1[:],
                            rhs1v[:, y0 : y0 + 4, 0:128],
                            start=True,
                            stop=True,
                        )
                    if g == 0 and k % (16 // CHUNKS) == 0 and k // (16 // CHUNKS) < CHUNKS - 1:
                        im2col_chunk(k // (16 // CHUNKS) + 1)
                    if g == 0 and k == 2:
                        # remaining weights (needed from conv3 onward)
                        t_l3 = wp.tile([128, 9 * 128], BF16)
                        nc.scalar.dma_start(out=t_l3[:], in_=lhsT3.ap())
                        t_wfc = wp.tile([128, 2560], BF16)
                        nc.scalar.dma_start(out=t_wfc[:], in_=wfc.ap())
                    Y0 = k * 4
                    pool_psum(
                        ps, 128, pv[:, Y0 + 1 : Y0 + 5, 1:65], 0, f"c1_{g}_{k}"
                    )
                pp1_tiles.append(pp1)

            # =======================  conv2  =======================
            pp2_tiles = []
            for q in range(4):  # image pairs
                g, pr = divmod(q, 2)
                pp1 = pp1_tiles[g]
                p1pitch = pp1.ap[0][0]
                rhs2 = []
                for j in range(2):  # imgs 2q+j; within-group index pr*2+j
                    i1 = pr * 2 + j
                    r2 = rhs2p.tile([96, 66 * 66], BF16, tag="rhs2", name=f"r2_{q}_{j}")
                    r2pitch = r2.ap[0][0]
                    # one DMA per kx: dst partitions p = 3c + kx
                    for kx in range(3):
                        src = bass.AP(
                            pp1.tensor,
                            pp1.offset + (32 * i1) * p1pitch + kx,
                            [[p1pitch, 32], [1, 66 * 66]],
                        )
                        dst = bass.AP(
                            r2.tensor,
                            r2.offset + kx * r2pitch,
                            [[3 * r2pitch, 32], [1, 66 * 66]],
                        )
                        (nc.sync if (j + kx) % 2 == 0 else nc.gpsimd).dma_start(
                            out=dst, in_=src
                        )
                    if dbg and q == 0 and j == 0:
                        d_rhs2 = nc.dram_tensor(
                            "d_rhs2", [96, 66 * 66], BF16, kind="ExternalOutput"
                        )
                        nc.sync.dma_start(out=d_rhs2.ap(), in_=r2[:])
                    rhs2.append(r2.rearrange("p (r q) -> p r q", q=66))

                pp2 = pp2p.tile([128, P2F], BF16, tag="pp2", name=f"pp2_{q}")
                pv2 = pp2.rearrange("p (r q) -> p r q", q=P2)
                nc.gpsimd.memset(pp2[:, 0:P2], 0)
                nc.gpsimd.memset(pp2[:, 33 * P2 : P2F], 0)
                nc.gpsimd.memset(pv2[:, 1:33, 0:1], 0)
                nc.gpsimd.memset(pv2[:, 1:33, 33:34], 0)

                for k in range(4):
                    ps = psp.tile([128, 1024], F32, tag="ps", name=f"ps2_{q}_{k}")
                    for h in range(2):
                        Y0 = k * 16 + h * 8
                        for ky in range(3):
                            for j in range(2):
                                nc.tensor.matmul(
                                    ps[64 * j : 64 * j + 64, h * 512 : (h + 1) * 512],
                                    t_l2[:, ky * 128 + 64 * j : ky * 128 + 64 * j + 64],
                                    rhs2[j][:, Y0 + ky : Y0 + ky + 8, 0:64],
                                    start=(ky == 0),
                                    stop=(ky == 2),
                                )
                    Y0 = k * 8
                    pool_psum(
                        ps, 64, pv2[:, Y0 + 1 : Y0 + 9, 1:33], 1, f"c2_{q}_{k}"
                    )
                pp2_tiles.append(pp2)

            # =======================  conv3  =======================
            for q in range(4):
                pv2 = pp2_tiles[q].rearrange("p (r q) -> p r q", q=P2)
                ps_ab = [
                    psp.tile([128, 1024], F32, tag="ps", name=f"ps3_{q}_{jj}")
                    for jj in range(2)
                ]
                for h in range(2):
                    Y0 = h * 16
                    for t in range(9):
                        a, b = divmod(t, 3)
                        for j in range(2):  # img A (rows 0-63), img B (rows 64-127)
                            nc.tensor.matmul(
                                ps_ab[j][:, h * 512 : (h + 1) * 512],
                                t_l3[64 * j : 64 * j + 64, t * 128 : (t + 1) * 128],
                                pv2[64 * j : 64 * j + 64, Y0 + a : Y0 + a + 16, b : b + 32],
                                start=(t == 0),
                                stop=(t == 8),
                            )
                for j in range(2):
                    img = 2 * q + j
                    xv = x_all.rearrange("p (i q) -> p i q", q=256)
                    ov = xv[:, img, :].rearrange("p (y x) -> p y x", x=16)
                    pool_psum(ps_ab[j], 32, ov, 2, f"c3_{q}_{j}")

            if dbg:
                d_pp1 = nc.dram_tensor("d_pp1", [128, P1F], BF16, kind="ExternalOutput")
                nc.sync.dma_start(out=d_pp1.ap(), in_=pp1_tiles[0][:])
                d_pp2 = nc.dram_tensor("d_pp2", [128, P2F], BF16, kind="ExternalOutput")
                nc.sync.dma_start(out=d_pp2.ap(), in_=pp2_tiles[0][:])
                d_xall = nc.dram_tensor("d_xall", [128, 2048], BF16, kind="ExternalOutput")
                nc.sync.dma_start(out=d_xall.ap(), in_=x_all[:])

            # =======================  fc  =======================
            ps_fc = psp.tile([128, 8], F32, tag="ps", name="ps_fc")
            xv = x_all.rearrange("p (i q) -> p i q", q=256)
            for p in range(256):
                cg = p % 4
                nc.tensor.matmul(
                    ps_fc[32 * cg : 32 * cg + 10, :],
                    t_wfc[:, 10 * p : 10 * p + 10],
                    xv[:, :, p],
                    start=(p < 4),
                    stop=(p >= 252),
                    tile_position=(0, 32 * cg),
                )
            sc0 = wp.tile([10, 8], F32)
            nc.scalar.activation(sc0[:], ps_fc[0:10, :], Copy)
            sc1 = wp.tile([10, 8], F32)
            nc.vector.tensor_add(sc1[:], ps_fc[32:42, :], sc0[:])
            sc2 = wp.tile([10, 8], F32)
            nc.vector.tensor_add(sc2[:], ps_fc[64:74, :], sc1[:])
            sc3 = wp.tile([10, 8], F32)
            nc.vector.tensor_add(sc3[:], ps_fc[96:106, :], sc2[:])
            sc = wp.tile([10, 8], F32)
            nc.scalar.activation(sc[:], sc3[:], Ident, bias=t_b[0:10, 3:4])
            nc.sync.dma_start(out=scores.ap(), in_=sc[:])

    nc.compile()
    return nc


def _prep_weights(w1, b1, w2, b2, w3, b3, w_fc, b_fc):
    """Host-side weight rearrangement (shared across cores)."""
    # conv1 block-diagonal lhsT: row (img*3+c)*9 + 3a+b, col m = img*32 + f
    l1 = np.zeros((108, 128), np.float32)
    for t in range(9):
        a, b = divmod(t, 3)
        for img in range(4):
            for c in range(3):
                l1[(img * 3 + c) * 9 + t, img * 32 : img * 32 + 32] = w1[:, c, a, b]
    # conv2: rows p = 3c + kx, col block ky: [W2_ky | W2_ky]
    l2 = np.zeros((96, 3 * 128), np.float32)
    for ky in range(3):
        for kx in range(3):
            blk = w2[:, :, ky, kx].T  # [c, f]
            l2[kx : 96 : 3, ky * 128 : ky * 128 + 64] = blk
            l2[kx : 96 : 3, ky * 128 + 64 : ky * 128 + 128] = blk
    # conv3: rows c (dup at 64+c), col block t
    l3 = np.zeros((128, 9 * 128), np.float32)
    for t in range(9):
        a, b = divmod(t, 3)
        blk = w3[:, :, a, b].T  # [c=64, f=128]
        l3[0:64, t * 128 : (t + 1) * 128] = blk
        l3[64:128, t * 128 : (t + 1) * 128] = blk
    # fc: w_fc[c*256 + p, cls] -> wfc[c, p*10 + cls]
    wf = np.ascontiguousarray(w_fc.reshape(128, 256, 10).reshape(128, 2560))
    # biases, per-partition layout matching each stage's PSUM partitions
    bia = np.zeros((128, 4), np.float32)
    bia[:, 0] = np.tile(np.asarray(b1, np.float32), 4)
    bia[:, 1] = np.tile(np.asarray(b2, np.float32), 2)
    bia[:, 2] = np.asarray(b3, np.float32)
    bia[0:10, 3] = np.asarray(b_fc, np.float32)
    return {
        "lhsT1": l1.astype(NPBF16),
        "lhsT2": l2.astype(NPBF16),
        "lhsT3": l3.astype(NPBF16),
        "wfc": wf.astype(NPBF16),
        "biases": bia,
    }


_NC_CACHE = {}


def get_nc():
    if "nc" not in _NC_CACHE:
        _NC_CACHE["nc"] = _build_nc()
    return _NC_CACHE["nc"]


def kernel(x, w1, b1, w2, b2, w3, b3, w_fc, b_fc, **run_kwargs):
    x = np.asarray(x, np.float32)
    wts = _prep_weights(
        np.asarray(w1, np.float32), np.asarray(b1, np.float32),
        np.asarray(w2, np.float32), np.asarray(b2, np.float32),
        np.asarray(w3, np.float32), np.asarray(b3, np.float32),
        np.asarray(w_fc, np.float32), np.asarray(b_fc, np.float32),
    )
    xpad = np.pad(x, ((0, 0), (0, 0), (1, 1), (1, 1))).astype(NPBF16)
    in_maps = []
    for core in range(N_CORES):
        m = dict(wts)
        m["xp"] = np.ascontiguousarray(xpad[core * IMGS : (core + 1) * IMGS]).reshape(-1)
        in_maps.append(m)

    nc = get_nc()
    res = run_bass_kernel_spmd(nc, in_maps, core_ids=list(range(N_CORES)), **run_kwargs)
    out = np.concatenate([r["scores"].T for r in res.results], axis=0)
    kernel.last_results = res
    return out.astype(np.float32)
